# revision 25
# baseline (speedup 1.0000x reference)
"""CAPMemory loss kernel for 8 Trainium2 NeuronCores.

Sharding: camera-sharded -- core c owns memory[c], the batch is replicated
(the per-sample stats each core produces are tiny, so this moves 16x less
HBM traffic than batch-sharding the replicated 128 MiB memory bank).

Device, per core (fp8 e4m3 DoubleRow matmul, fp32 PSUM):
  S[b, l] = <x_norm[b], memory[c, l]> * FP8_SCALE^2      [1024, 2048]
  E       = exp(S / (FP8_SCALE^2 * T))  (ACT, bf16)
  zin[b]  = sum_l E[b, l]               (ACT free-dim accumulate, fp32)
  cand    = top-8 of each 256-wide chunk of E -> 64 values/sample (DVE MAX8)

Host merge:
  epos[c, b] = exp(<x8[b], m8[c, tgt_b]>/T') recomputed in f32 from the
  exact fp8 operands the device consumed; intra CE = log(zin) - log(epos)
  on the own-camera core. For the inter loss the positive's value is
  removed from its camera's candidate list (nearest match to epos), the
  8x64 candidates are merged, and the exact top-50 negatives feed the
  log-sum-exp. A global top-50 element can only be missing from the
  candidates if >=8 larger elements share its 256-chunk (P ~ 1e-5 per
  run, and the substitute is the next-ranked value, so the effect is
  ~1e-6 relative even then).
"""

import numpy as np

T = 0.05
HARD_NEG_K = 50
LOSS_WEIGHT = 0.5
N_CAMS = 8
L = 2048
D = 2048
B = 1024
NBT = 8          # batch tiles of 128
KC8 = 8          # contraction chunks of 256 (fp8 DoubleRow: 2 k-rows/cell)
FP8_SCALE = 32.0  # pre-scale before e4m3 cast (keeps values out of denormals)
NCH = 8          # candidate chunks per row
CHW = 256        # chunk width
NTOP = NCH * 8   # candidates shipped per camera (top-8 of each chunk)

_CACHE = {}


def _split_multi_waits(nc):
    """This container's walrus build rejects instructions carrying more than
    one sync wait ('Too many sync wait commands'). Hoist all but the last
    wait of each instruction onto same-engine Drain carriers placed just
    before it — semantically identical on an in-order engine stream."""
    import concourse.mybir as mybir

    n = 0
    for fn in nc.m.functions:
        for bb in fn.blocks:
            out = []
            for inst in bb.instructions:
                si = inst.sync_info
                if si is not None and si.on_wait and len(si.on_wait) > 1:
                    waits = list(si.on_wait)
                    for w in waits[:-1]:
                        d = mybir.InstDrain(name=f"ws-{n}", ins=[], outs=[])
                        n += 1
                        d.engine = inst.engine
                        d.sync_info = mybir.SyncInfo(on_wait=[w], on_update=[])
                        out.append(d)
                    si.on_wait = [waits[-1]]
                out.append(inst)
            if n:
                bb.instructions = out


def _build():
    import concourse.bass as bass
    import concourse.mybir as mybir
    from concourse import tile

    f32 = mybir.dt.float32
    bf16 = mybir.dt.bfloat16
    f8 = mybir.dt.float8e4
    Act = mybir.ActivationFunctionType

    nc = bass.Bass()
    xT = nc.dram_tensor("xT", [KC8, 128, 2, B], f8, kind="ExternalInput")
    mT = nc.dram_tensor("mT", [KC8, 128, 2, L], f8, kind="ExternalInput")
    zin_d = nc.dram_tensor("zin", [128, NBT], f32, kind="ExternalOutput")
    topv_d = nc.dram_tensor("topv", [NBT, 128, NTOP], bf16, kind="ExternalOutput")

    with tile.TileContext(nc) as tc:
        with (
            tc.tile_pool(name="const", bufs=1) as cpool,
            tc.tile_pool(name="psum", bufs=2, space="PSUM") as ppool,
            tc.tile_pool(name="work", bufs=2) as wpool,
            tc.tile_pool(name="small", bufs=2) as spool,
        ):
            X = cpool.tile([128, KC8, 2, B], f8)
            M = cpool.tile([128, KC8, 2, L], f8)
            for kc in range(KC8):
                nc.sync.dma_start(X[:, kc, :, :], xT[kc])
                nc.sync.dma_start(M[:, kc, :, :], mT[kc])
            ZIN = cpool.tile([128, NBT], f32)
            ZP = cpool.tile([128, 4], f32)

            for bt in range(NBT):
                S = ppool.tile([128, L], f32, tag="S")
                # rotate kc order per btile so later btiles don't all stall
                # on the last-arriving mT chunk
                kcs = [(kc + bt) % KC8 for kc in range(KC8)]
                for i, kc in enumerate(kcs):
                    for nch in range(4):
                        nc.tensor.matmul(
                            S[:, nch * 512 : (nch + 1) * 512],
                            X[:, kc, :, bt * 128 : (bt + 1) * 128],
                            M[:, kc, :, nch * 512 : (nch + 1) * 512],
                            start=(i == 0),
                            stop=(i == KC8 - 1),
                            perf_mode=mybir.MatmulPerfMode.DoubleRow,
                        )
                # last btile: exp per 512-bank so candidate chunks overlap
                # the exp instead of serializing after one big EXP
                E = wpool.tile([128, L], bf16, tag="E")
                if bt == NBT - 1:
                    for nch in range(4):
                        sl = slice(nch * 512, (nch + 1) * 512)
                        nc.scalar.activation(
                            E[:, sl], S[:, sl], Act.Exp,
                            scale=1.0 / (FP8_SCALE * FP8_SCALE * T),
                            accum_out=ZP[:, nch : nch + 1],
                        )
                    nc.vector.reduce_sum(
                        ZIN[:, bt : bt + 1], ZP[:], axis=mybir.AxisListType.X
                    )
                else:
                    nc.scalar.activation(
                        E[:], S[:], Act.Exp,
                        scale=1.0 / (FP8_SCALE * FP8_SCALE * T),
                        accum_out=ZIN[:, bt : bt + 1],
                    )
                # top-8 of each 256-chunk of raw E -> 64 candidates/camera;
                # host merges the exact global top-50 (the positive's value
                # is removed host-side by near-match against epos)
                cand = spool.tile([128, NCH * 8], bf16, tag="cand")
                for ch in range(NCH):
                    nc.vector.max(
                        cand[:, ch * 8 : (ch + 1) * 8],
                        E[:, ch * CHW : (ch + 1) * CHW],
                    )
                nc.sync.dma_start(topv_d[bt], cand[:])

            nc.sync.dma_start(zin_d[:], ZIN[:])

    _split_multi_waits(nc)
    return nc


def _get_nc():
    if "nc" not in _CACHE:
        _CACHE["nc"] = _build()
    return _CACHE["nc"]


def _pack_fp8(aT, ncols, f8):
    # [D, n] -> [KC8, 128, 2, n] with d = kc*256 + j*128 + p
    v = np.clip(aT * FP8_SCALE, -240.0, 240.0)
    v = v.reshape(KC8, 2, 128, ncols).transpose(0, 2, 1, 3)
    return np.ascontiguousarray(v).astype(f8)


def _prepare_in_maps(inputs, memory):
    import ml_dtypes

    f8 = ml_dtypes.float8_e4m3
    inputs = np.asarray(inputs, np.float32)
    memory = np.asarray(memory, np.float32)
    x = inputs / np.linalg.norm(inputs, axis=1, keepdims=True)
    xT = _pack_fp8(x.T, B, f8)
    in_maps = []
    for c in range(N_CAMS):
        mT = _pack_fp8(memory[c].T, L, f8)
        in_maps.append({"xT": xT, "mT": mT})
    return in_maps


def kernel(inputs, memory, indexes, cams_all, labels_all):
    from concourse.bass_utils import run_bass_kernel_spmd

    indexes = np.asarray(indexes).astype(np.int64)
    cams_all = np.asarray(cams_all).astype(np.int64)
    labels_all = np.asarray(labels_all).astype(np.int64)
    cams = cams_all[indexes]

    in_maps = _prepare_in_maps(inputs, memory)
    nc = _get_nc()
    res = run_bass_kernel_spmd(nc, in_maps, list(range(N_CAMS)))

    # epos = exp(S[t]/T) computed host-side from the same fp8-quantized
    # inputs the device consumed (f32 arithmetic ~= PSUM fp32 accumulate)
    tgts = labels_all[indexes]
    x8 = in_maps[0]["xT"].transpose(0, 2, 1, 3).reshape(D, B).astype(np.float32)
    epos = np.empty((N_CAMS, B), np.float64)
    for c in range(N_CAMS):
        m8 = in_maps[c]["mT"].transpose(0, 2, 1, 3).reshape(D, L).astype(np.float32)
        mt = m8[:, tgts]                     # [D, B]
        s_t = np.einsum("db,db->b", x8, mt, optimize=True)
        epos[c] = np.exp(s_t.astype(np.float64) / (FP8_SCALE * FP8_SCALE * T))

    # gather per-core stats; [128, NBT] -> [B] with b = bt*128 + p
    zin = np.empty((N_CAMS, B), np.float64)
    topv = np.empty((N_CAMS, B, NTOP), np.float64)
    for c in range(N_CAMS):
        r = res.results[c]
        zin[c] = r["zin"].astype(np.float64).T.reshape(B)
        topv[c] = r["topv"].astype(np.float64).reshape(B, NTOP)

    # ---- intra: CE against own camera, mean within camera group, summed
    bidx = np.arange(B)
    bidx_all = bidx
    zin_own = zin[cams, bidx]
    epos_own = epos[cams, bidx]
    ce = np.log(zin_own) - np.log(epos_own)
    cnt = np.bincount(cams, minlength=N_CAMS).astype(np.float64)
    ce_sum = np.bincount(cams, weights=ce, minlength=N_CAMS)
    loss_intra = np.sum(ce_sum / np.maximum(cnt, 1.0))

    # remove the positive's own value from each camera's candidate list:
    # nearest candidate within 0.5% of the host-computed epos (device values
    # are bf16-rounded, so exact equality is not available)
    for c in range(N_CAMS):
        relerr = np.abs(topv[c] - epos[c][:, None]) / epos[c][:, None]
        j = np.argmin(relerr, axis=1)
        hit = relerr[bidx_all, j] < 5e-3
        topv[c][bidx_all[hit], j[hit]] = 0.0

    # ---- inter: exact global top-50 negatives from 8x56 candidates
    cand = topv[:, bidx, :].transpose(1, 0, 2).reshape(B, N_CAMS * NTOP)
    part = np.partition(cand, cand.shape[1] - HARD_NEG_K, axis=1)
    z50 = part[:, cand.shape[1] - HARD_NEG_K :].sum(axis=1)
    sum_epos = epos[:, bidx].sum(axis=0)
    lse = np.log(sum_epos + z50)
    mean_logpos = np.log(epos[:, bidx]).mean(axis=0)
    per_sample = lse - mean_logpos
    inter_sum = np.bincount(cams, weights=per_sample, minlength=N_CAMS)
    loss_inter = np.sum(inter_sum / np.maximum(cnt, 1.0)) * LOSS_WEIGHT

    return np.float32(loss_intra), np.float32(loss_inter)


# revision 26
# speedup vs baseline: 1.0076x; 1.0076x over previous
"""CAPMemory loss kernel for 8 Trainium2 NeuronCores.

Sharding: camera-sharded -- core c owns memory[c], the batch is replicated
(the per-sample stats each core produces are tiny, so this moves 16x less
HBM traffic than batch-sharding the replicated 128 MiB memory bank).

Device, per core (fp8 e4m3 DoubleRow matmul, fp32 PSUM):
  S[b, l] = <x_norm[b], memory[c, l]> * FP8_SCALE^2      [1024, 2048]
  E       = exp(S / (FP8_SCALE^2 * T))  (ACT, bf16)
  zin[b]  = sum_l E[b, l]               (ACT free-dim accumulate, fp32)
  cand    = top-8 of each 256-wide chunk of E -> 64 values/sample (DVE MAX8)

Host merge:
  epos[c, b] = exp(<x8[b], m8[c, tgt_b]>/T') recomputed in f32 from the
  exact fp8 operands the device consumed; intra CE = log(zin) - log(epos)
  on the own-camera core. For the inter loss the positive's value is
  removed from its camera's candidate list (nearest match to epos), the
  8x64 candidates are merged, and the exact top-50 negatives feed the
  log-sum-exp. A global top-50 element can only be missing from the
  candidates if >=8 larger elements share its 256-chunk (P ~ 1e-5 per
  run, and the substitute is the next-ranked value, so the effect is
  ~1e-6 relative even then).
"""

import numpy as np

T = 0.05
HARD_NEG_K = 50
LOSS_WEIGHT = 0.5
N_CAMS = 8
L = 2048
D = 2048
B = 1024
NBT = 8          # batch tiles of 128
KC8 = 8          # contraction chunks of 256 (fp8 DoubleRow: 2 k-rows/cell)
FP8_SCALE = 32.0  # pre-scale before e4m3 cast (keeps values out of denormals)
NCH = 8          # candidate chunks per row
CHW = 256        # chunk width
NTOP = NCH * 8   # candidates shipped per camera (top-8 of each chunk)

_CACHE = {}


def _split_multi_waits(nc):
    """This container's walrus build rejects instructions carrying more than
    one sync wait ('Too many sync wait commands'). Hoist all but the last
    wait of each instruction onto same-engine Drain carriers placed just
    before it — semantically identical on an in-order engine stream."""
    import concourse.mybir as mybir

    n = 0
    for fn in nc.m.functions:
        for bb in fn.blocks:
            out = []
            for inst in bb.instructions:
                si = inst.sync_info
                if si is not None and si.on_wait and len(si.on_wait) > 1:
                    waits = list(si.on_wait)
                    for w in waits[:-1]:
                        d = mybir.InstDrain(name=f"ws-{n}", ins=[], outs=[])
                        n += 1
                        d.engine = inst.engine
                        d.sync_info = mybir.SyncInfo(on_wait=[w], on_update=[])
                        out.append(d)
                    si.on_wait = [waits[-1]]
                out.append(inst)
            if n:
                bb.instructions = out


def _build():
    import concourse.bass as bass
    import concourse.mybir as mybir
    from concourse import tile

    f32 = mybir.dt.float32
    bf16 = mybir.dt.bfloat16
    f8 = mybir.dt.float8e4
    Act = mybir.ActivationFunctionType

    nc = bass.Bass()
    xT = nc.dram_tensor("xT", [KC8, 128, 2, B], f8, kind="ExternalInput")
    mT = nc.dram_tensor("mT", [KC8, 128, 2, L], f8, kind="ExternalInput")
    zin_d = nc.dram_tensor("zin", [128, NBT], f32, kind="ExternalOutput")
    topv_d = nc.dram_tensor("topv", [NBT, 128, NTOP], bf16, kind="ExternalOutput")

    with tile.TileContext(nc) as tc:
        with (
            tc.tile_pool(name="const", bufs=1) as cpool,
            tc.tile_pool(name="psum", bufs=2, space="PSUM") as ppool,
            tc.tile_pool(name="work", bufs=2) as wpool,
            tc.tile_pool(name="small", bufs=2) as spool,
        ):
            X = cpool.tile([128, KC8, 2, B], f8)
            M = cpool.tile([128, KC8, 2, L], f8)
            for kc in range(KC8):
                nc.sync.dma_start(X[:, kc, :, :], xT[kc])
                nc.sync.dma_start(M[:, kc, :, :], mT[kc])
            ZIN = cpool.tile([128, NBT], f32)
            ZP = cpool.tile([128, 4], f32)

            # PE warm-up: HAM needs ~3-4us of sustained activity to reach
            # 2.4 GHz. Run throwaway matmuls on a zeroed scratch tile while
            # the input DMAs are still in flight, so the first real matmuls
            # start at full clock instead of paying the cold-clock penalty.
            GB = cpool.tile([128, 640], f8)
            nc.vector.memset(GB[:], 0.0)
            WARM = ppool.tile([128, 512], f32, tag="S")
            for _ in range(10):
                nc.tensor.matmul(
                    WARM[:], GB[:, 0:128], GB[:, 128:640],
                    start=True, stop=True,
                )

            for bt in range(NBT):
                S = ppool.tile([128, L], f32, tag="S")
                # rotate kc order per btile so later btiles don't all stall
                # on the last-arriving mT chunk
                kcs = [(kc + bt) % KC8 for kc in range(KC8)]
                for i, kc in enumerate(kcs):
                    for nch in range(4):
                        nc.tensor.matmul(
                            S[:, nch * 512 : (nch + 1) * 512],
                            X[:, kc, :, bt * 128 : (bt + 1) * 128],
                            M[:, kc, :, nch * 512 : (nch + 1) * 512],
                            start=(i == 0),
                            stop=(i == KC8 - 1),
                            perf_mode=mybir.MatmulPerfMode.DoubleRow,
                        )
                # last btile: exp per 512-bank so candidate chunks overlap
                # the exp instead of serializing after one big EXP
                E = wpool.tile([128, L], bf16, tag="E")
                if bt == NBT - 1:
                    for nch in range(4):
                        sl = slice(nch * 512, (nch + 1) * 512)
                        nc.scalar.activation(
                            E[:, sl], S[:, sl], Act.Exp,
                            scale=1.0 / (FP8_SCALE * FP8_SCALE * T),
                            accum_out=ZP[:, nch : nch + 1],
                        )
                    nc.vector.reduce_sum(
                        ZIN[:, bt : bt + 1], ZP[:], axis=mybir.AxisListType.X
                    )
                else:
                    nc.scalar.activation(
                        E[:], S[:], Act.Exp,
                        scale=1.0 / (FP8_SCALE * FP8_SCALE * T),
                        accum_out=ZIN[:, bt : bt + 1],
                    )
                # top-8 of each 256-chunk of raw E -> 64 candidates/camera;
                # host merges the exact global top-50 (the positive's value
                # is removed host-side by near-match against epos)
                cand = spool.tile([128, NCH * 8], bf16, tag="cand")
                for ch in range(NCH):
                    nc.vector.max(
                        cand[:, ch * 8 : (ch + 1) * 8],
                        E[:, ch * CHW : (ch + 1) * CHW],
                    )
                nc.sync.dma_start(topv_d[bt], cand[:])

            nc.sync.dma_start(zin_d[:], ZIN[:])

    _split_multi_waits(nc)
    return nc


def _get_nc():
    if "nc" not in _CACHE:
        _CACHE["nc"] = _build()
    return _CACHE["nc"]


def _pack_fp8(aT, ncols, f8):
    # [D, n] -> [KC8, 128, 2, n] with d = kc*256 + j*128 + p
    v = np.clip(aT * FP8_SCALE, -240.0, 240.0)
    v = v.reshape(KC8, 2, 128, ncols).transpose(0, 2, 1, 3)
    return np.ascontiguousarray(v).astype(f8)


def _prepare_in_maps(inputs, memory):
    import ml_dtypes

    f8 = ml_dtypes.float8_e4m3
    inputs = np.asarray(inputs, np.float32)
    memory = np.asarray(memory, np.float32)
    x = inputs / np.linalg.norm(inputs, axis=1, keepdims=True)
    xT = _pack_fp8(x.T, B, f8)
    in_maps = []
    for c in range(N_CAMS):
        mT = _pack_fp8(memory[c].T, L, f8)
        in_maps.append({"xT": xT, "mT": mT})
    return in_maps


def kernel(inputs, memory, indexes, cams_all, labels_all):
    from concourse.bass_utils import run_bass_kernel_spmd

    indexes = np.asarray(indexes).astype(np.int64)
    cams_all = np.asarray(cams_all).astype(np.int64)
    labels_all = np.asarray(labels_all).astype(np.int64)
    cams = cams_all[indexes]

    in_maps = _prepare_in_maps(inputs, memory)
    nc = _get_nc()
    res = run_bass_kernel_spmd(nc, in_maps, list(range(N_CAMS)))

    # epos = exp(S[t]/T) computed host-side from the same fp8-quantized
    # inputs the device consumed (f32 arithmetic ~= PSUM fp32 accumulate)
    tgts = labels_all[indexes]
    x8 = in_maps[0]["xT"].transpose(0, 2, 1, 3).reshape(D, B).astype(np.float32)
    epos = np.empty((N_CAMS, B), np.float64)
    for c in range(N_CAMS):
        m8 = in_maps[c]["mT"].transpose(0, 2, 1, 3).reshape(D, L).astype(np.float32)
        mt = m8[:, tgts]                     # [D, B]
        s_t = np.einsum("db,db->b", x8, mt, optimize=True)
        epos[c] = np.exp(s_t.astype(np.float64) / (FP8_SCALE * FP8_SCALE * T))

    # gather per-core stats; [128, NBT] -> [B] with b = bt*128 + p
    zin = np.empty((N_CAMS, B), np.float64)
    topv = np.empty((N_CAMS, B, NTOP), np.float64)
    for c in range(N_CAMS):
        r = res.results[c]
        zin[c] = r["zin"].astype(np.float64).T.reshape(B)
        topv[c] = r["topv"].astype(np.float64).reshape(B, NTOP)

    # ---- intra: CE against own camera, mean within camera group, summed
    bidx = np.arange(B)
    bidx_all = bidx
    zin_own = zin[cams, bidx]
    epos_own = epos[cams, bidx]
    ce = np.log(zin_own) - np.log(epos_own)
    cnt = np.bincount(cams, minlength=N_CAMS).astype(np.float64)
    ce_sum = np.bincount(cams, weights=ce, minlength=N_CAMS)
    loss_intra = np.sum(ce_sum / np.maximum(cnt, 1.0))

    # remove the positive's own value from each camera's candidate list:
    # nearest candidate within 0.5% of the host-computed epos (device values
    # are bf16-rounded, so exact equality is not available)
    for c in range(N_CAMS):
        relerr = np.abs(topv[c] - epos[c][:, None]) / epos[c][:, None]
        j = np.argmin(relerr, axis=1)
        hit = relerr[bidx_all, j] < 5e-3
        topv[c][bidx_all[hit], j[hit]] = 0.0

    # ---- inter: exact global top-50 negatives from 8x56 candidates
    cand = topv[:, bidx, :].transpose(1, 0, 2).reshape(B, N_CAMS * NTOP)
    part = np.partition(cand, cand.shape[1] - HARD_NEG_K, axis=1)
    z50 = part[:, cand.shape[1] - HARD_NEG_K :].sum(axis=1)
    sum_epos = epos[:, bidx].sum(axis=0)
    lse = np.log(sum_epos + z50)
    mean_logpos = np.log(epos[:, bidx]).mean(axis=0)
    per_sample = lse - mean_logpos
    inter_sum = np.bincount(cams, weights=per_sample, minlength=N_CAMS)
    loss_inter = np.sum(inter_sum / np.maximum(cnt, 1.0)) * LOSS_WEIGHT

    return np.float32(loss_intra), np.float32(loss_inter)
